# revision 43
# baseline (speedup 1.0000x reference)
"""DropSphereNd Trainium2 kernel.

Full computation (per sample n, channels c):
    activ = embeds @ table                      # [n, c]
    t     = 17th-smallest(activ, axis=1)        # [n, 1]
    out   = x * (activ >= t) * c/(c-16)

Sharding: data-parallel over batch n across 8 cores (x/embeds sharded,
table replicated).  Per core: x shard [8, 256, 56, 56] viewed as
[2048, 3136]; the mask is computed on-device (tiny matmul + iterative
min-extraction) and applied as a per-partition scalar multiply while
streaming x through SBUF.

The kernel is DMA-bound: 25.7 MB in + 25.7 MB out per core.  Reads and
writes get partially independent bandwidth (duplex AXI/HBM paths), so
the schedule maximizes read+write overlap and keeps the store queue
backlogged.  Schedule: 16 tiles of [128, 3136] (half a sample each,
contiguous 1.6 MB DRAM blocks).  Loads ride TWO queues (SP HWDGE +
gpsimd SWDGE, alternating tiles) while stores ride one (ACT HWDGE):
loads keep ~2/3 of the queue presence, finish early, and the ACT store
stream runs with a deep backlog mid-kernel, then saturates the tail
with no mul-latency bubble.  Tile 14's load is split 3:1 between the
two load queues: SP otherwise finishes ~5 us after POOL (the HWDGE
store ring contends with SP more than with SWDGE), and balancing the
finish times starts the final store drain earlier.  Tiny mask inputs
load at the head of the SP ring; the embeds transpose happens on PE (a
transposed 128x4B-descriptor DMA would starve ~10 us behind bulk
traffic).  Rejected on HW: fp16 staging, smooth early-store schedules,
dual store queues, big/strided tiles, contiguous-half and
partition-half load splits (the two-read-queue interleave penalty is
SDMA-engine-internal and layout-independent).

Raw bass (no Tile): the pinned walrus codegen allows only ONE sync-wait
per compute instruction, so all cross-engine deps use standalone
wait_ge sequencer commands.

Engine plan:
  SP   (nc.sync)   - small input DMAs, then x loads (even tiles)
  POOL (nc.gpsimd) - x loads, odd tiles
  ACT  (nc.scalar) - output DMAs
  PE   (nc.tensor) - embeds transpose + projection + 2 mask transposes
  DVE  (nc.vector) - threshold search, mask build, streaming multiplies
"""

import sys

if "/opt/trn_rl_repo" not in sys.path:
    sys.path.insert(0, "/opt/trn_rl_repo")

from contextlib import ExitStack

import numpy as np

import concourse.bass as bass
from concourse import mybir
from concourse.bass_utils import run_bass_kernel_spmd

N, C, H, W = 64, 256, 56, 56
HW = H * W  # 3136
E = 16
NCORES = 8
NLOC = N // NCORES  # 8 samples per core
INDEX = 16  # ceil(C ** 0.5)
SCALE = float(C) / (C - INDEX)
F32 = mybir.dt.float32
NT = 16  # tiles: [128, HW], tile k = sample k//2, channels (k%2)*128+p
SLOTS = 14  # x-tile ring slots (12.25 KB/partition each)
SPLIT = 2080  # tile-14 free-axis split point (SP:POOL load rebalance;
# 2352 left a residual 2.1 us SP lag -- 270 more columns closes it)

_NC_CACHE = {}


def _build_nc() -> bass.Bass:
    # detect_race_conditions only affects the interpreter: its raw-bass model
    # has no same-engine program-order edges, so every chained DVE op would be
    # flagged.  Cross-engine ordering is handled by the explicit sems below.
    nc = bass.Bass(detect_race_conditions=False)
    x = nc.dram_tensor("x", [NLOC * C, HW], F32, kind="ExternalInput")
    emb = nc.dram_tensor("embeds", [NLOC, E], F32, kind="ExternalInput")
    tab = nc.dram_tensor("table", [E, C], F32, kind="ExternalInput")
    out = nc.dram_tensor("out", [NLOC * C, HW], F32, kind="ExternalOutput")
    ident_d = nc.inline_tensor(np.eye(NLOC, dtype=np.float32), name="ident8")

    # row r = k*128 + p  ->  sample k//2, channel (k%2)*128 + p
    x_k = x[:, :].rearrange("(k p) f -> k p f", p=128)
    o_k = out[:, :].rearrange("(k p) f -> k p f", p=128)

    with ExitStack() as ctx:
        sb = lambda name, shape: ctx.enter_context(nc.sbuf_tensor(name, shape, F32))
        ps = lambda name, shape: ctx.enter_context(nc.psum_tensor(name, shape, F32))

        tab_s = sb("tab_s", [E, C])
        emb_s = sb("emb_s", [NLOC, E])
        embT = sb("embT", [E, NLOC])
        ident = sb("ident", [NLOC, NLOC])
        v = sb("v", [NLOC, C])
        v2 = sb("v2", [NLOC, C])
        mx = sb("mx", [NLOC, 8])
        m = sb("m", [NLOC, C])
        mA = sb("mA", [C // 2, NLOC])  # channels   0-127 x sample
        mB = sb("mB", [C // 2, NLOC])  # channels 128-255 x sample
        xbuf = [sb(f"xbuf{i}", [128, HW]) for i in range(SLOTS)]

        embT_p = ps("embT_p", [E, NLOC])
        activ_p = ps("activ_p", [NLOC, C])
        mA_p = ps("mA_p", [C // 2, NLOC])
        mB_p = ps("mB_p", [C // 2, NLOC])

        ld = ctx.enter_context(nc.semaphore("ld"))
        eb = ctx.enter_context(nc.semaphore("eb"))
        fz = ctx.enter_context(nc.semaphore("fz"))
        dv = ctx.enter_context(nc.semaphore("dv"))
        pe = ctx.enter_context(nc.semaphore("pe"))
        # per-ring-slot DMA sems: same-sem increments are serialized by the
        # slot lifecycle, so wait values are unambiguous (race-detector clean)
        xs = [ctx.enter_context(nc.semaphore(f"xs{i}")) for i in range(SLOTS)]
        ss = [ctx.enter_context(nc.semaphore(f"ss{i}")) for i in range(SLOTS)]

        block = ctx.enter_context(nc.Block())

        # Smalls first on SP: ~25 contiguous descriptors drain in a couple
        # of SDMA round-robin visits even once bulk loads queue behind them
        # (a transposed-embeds DMA would be a 128x4B descriptor spray that
        # starves for 10+ us behind bulk traffic; PE transposes instead).
        # Even tiles then load on the same SP HWDGE ring.
        @block.sync
        def _(sync):
            sync.dma_start(out=tab_s[:, :], in_=tab[:, :]).then_inc(ld, 16)
            sync.dma_start(out=emb_s[:, :], in_=emb[:, :]).then_inc(ld, 16)
            sync.dma_start(out=ident[:, :], in_=ident_d[:, :]).then_inc(ld, 16)
            for k in range(0, NT - 2, 2):
                sync.dma_start(out=xbuf[k % SLOTS][:, :], in_=x_k[k]).then_inc(
                    xs[k % SLOTS], 16
                )
            # tile 14 is split 3:1 with POOL: SP consistently finishes
            # ~5 us after POOL (the HWDGE store ring contends with SP more
            # than with SWDGE), so ~0.4 MB shifts to POOL to balance the
            # two load queues' finish times
            sync.wait_ge(ss[0], 16)  # slot free once store of tile 0 drained
            sync.dma_start(
                out=xbuf[0][:, 0:SPLIT], in_=x_k[14][:, 0:SPLIT]
            ).then_inc(xs[0], 16)

        # Odd tiles load via SWDGE so loads occupy 2 of the 3 busy DMA
        # queues (bandwidth shares follow queue counts under the SDMA
        # packet round-robin).
        @block.gpsimd
        def _(gpsimd):
            for k in range(1, NT - 1, 2):
                gpsimd.dma_start(out=xbuf[k % SLOTS][:, :], in_=x_k[k]).then_inc(
                    xs[k % SLOTS], 16
                )
            gpsimd.wait_ge(ss[0], 16)
            gpsimd.dma_start(
                out=xbuf[0][:, SPLIT:HW], in_=x_k[14][:, SPLIT:HW]
            ).then_inc(xs[0], 16)
            gpsimd.wait_ge(ss[1], 16)
            gpsimd.dma_start(out=xbuf[1][:, :], in_=x_k[15]).then_inc(xs[1], 16)

        @block.tensor
        def _(tensor):
            tensor.wait_ge(ld, 48)  # tab_s + emb_s + ident resident
            tensor.matmul(
                embT_p[:, :], emb_s[:, :], ident[:, :], start=True, stop=True
            ).then_inc(pe, 1)
            tensor.wait_ge(eb, 1)  # embT copied to SBUF
            tensor.matmul(
                activ_p[:, :], embT[:, :], tab_s[:, :], start=True, stop=True
            ).then_inc(pe, 1)
            tensor.wait_ge(dv, 1)  # mask row built
            tensor.matmul(
                mA_p[:, :], m[:, 0 : C // 2], ident[:, :], start=True, stop=True
            ).then_inc(pe, 1)
            tensor.matmul(
                mB_p[:, :], m[:, C // 2 : C], ident[:, :], start=True, stop=True
            ).then_inc(pe, 1)

        # The 16 smallest of activ == the 16 largest of v = -activ.  DVE's
        # max (top-8 per partition) + match_replace (zap those 8) drop them
        # in two rounds; surviving lanes keep their value, zapped lanes hold
        # MINV, so the mask is one compare against an immediate.  No
        # data-dependent scalar operands anywhere: TensorScalarPtr fetches
        # its scalar at sequencer dispatch (ahead of the DVE pipe), so only
        # mA/mB -- real pointer operands of the streaming muls -- need a
        # sem fence.
        MINV = -1.0e30

        @block.vector
        def _(vector):
            vector.wait_ge(pe, 1)
            vector.tensor_copy(embT[:, :], embT_p[:, :]).then_inc(eb, 1)
            vector.wait_ge(pe, 2)
            vector.tensor_scalar_mul(v[:, :], activ_p[:, :], -1.0)
            # match_replace prefetches its 8-value table at dispatch, ahead
            # of the DVE pipe -- fence each max before consuming it
            vector.max(mx[:, :], v[:, :]).then_inc(fz, 1)
            vector.wait_ge(fz, 1)
            vector.match_replace(
                out=v2[:, :], in_to_replace=mx[:, :], in_values=v[:, :],
                imm_value=MINV,
            )
            vector.max(mx[:, :], v2[:, :]).then_inc(fz, 1)
            vector.wait_ge(fz, 2)
            vector.match_replace(
                out=v2[:, :], in_to_replace=mx[:, :], in_values=v2[:, :],
                imm_value=MINV,
            )
            # keep[c] <=> v2[c] != MINV ; mask = keep * SCALE
            # (immediate compare: real values are > MINV/2)
            vector.tensor_scalar(
                out=m[:, :],
                in0=v2[:, :],
                scalar1=MINV / 2,
                scalar2=SCALE,
                op0=mybir.AluOpType.is_ge,
                op1=mybir.AluOpType.mult,
            ).then_inc(dv, 1)
            vector.wait_ge(pe, 4)
            vector.tensor_copy(mA[:, :], mA_p[:, :])
            vector.tensor_copy(mB[:, :], mB_p[:, :]).then_inc(dv, 1)
            vector.wait_ge(dv, 2)  # mA/mB committed before mul ptr-fetches
            for k in range(NT):
                # slot 0 second pass needs 3 incs: load 0 + both tile-14 halves
                vector.wait_ge(xs[k % SLOTS], 48 if k == 14 else 16 * (k // SLOTS + 1))
                mcol = (mA if k % 2 == 0 else mB)[:, k // 2 : k // 2 + 1]
                vector.tensor_scalar_mul(
                    xbuf[k % SLOTS][:, :], xbuf[k % SLOTS][:, :], mcol
                ).then_inc(dv, 1)

        DV_BASE = 2  # dv value once masks + mA/mB copies are done

        @block.scalar
        def _(scalar):
            for k in range(NT):
                scalar.wait_ge(dv, DV_BASE + (k + 1))  # mul of tile k done
                scalar.dma_start(out=o_k[k], in_=xbuf[k % SLOTS][:, :]).then_inc(
                    ss[k % SLOTS], 16
                )

    return nc


def _get_nc() -> bass.Bass:
    if "nc" not in _NC_CACHE:
        _NC_CACHE["nc"] = _build_nc()
    return _NC_CACHE["nc"]


def _in_maps(x, embeds, table):
    x = np.ascontiguousarray(np.asarray(x, dtype=np.float32))
    embeds = np.ascontiguousarray(np.asarray(embeds, dtype=np.float32))
    table = np.ascontiguousarray(np.asarray(table, dtype=np.float32))
    maps = []
    for i in range(NCORES):
        maps.append(
            {
                "x": x[i * NLOC : (i + 1) * NLOC].reshape(NLOC * C, HW),
                "embeds": embeds[i * NLOC : (i + 1) * NLOC],
                "table": table,
            }
        )
    return maps


def kernel(x, embeds, table):
    nc = _get_nc()
    res = run_bass_kernel_spmd(nc, _in_maps(x, embeds, table), list(range(NCORES)))
    shards = [
        np.asarray(res.results[i]["out"]).reshape(NLOC, C, H, W)
        for i in range(NCORES)
    ]
    return np.concatenate(shards, axis=0)


def kernel_profiled(x, embeds, table, **trace_kwargs):
    """Same as kernel() but with NTFF tracing; returns (output, BassKernelResults)."""
    nc = _get_nc()
    res = run_bass_kernel_spmd(
        nc, _in_maps(x, embeds, table), list(range(NCORES)), trace=True, **trace_kwargs
    )
    shards = [
        np.asarray(res.results[i]["out"]).reshape(NLOC, C, H, W)
        for i in range(NCORES)
    ]
    return np.concatenate(shards, axis=0), res


# revision 44
# speedup vs baseline: 1.0800x; 1.0800x over previous
"""DropSphereNd Trainium2 kernel.

Full computation (per sample n, channels c):
    activ = embeds @ table                      # [n, c]
    t     = 17th-smallest(activ, axis=1)        # [n, 1]
    out   = x * (activ >= t) * c/(c-16)

Sharding: data-parallel over batch n across 8 cores (x/embeds sharded,
table replicated).  Per core: x shard [8, 256, 56, 56] viewed as
[2048, 3136]; the mask is computed on-device (tiny matmul + iterative
min-extraction) and applied as a per-partition scalar multiply while
streaming x through SBUF.

The kernel is DMA-bound: 25.7 MB in + 25.7 MB out per core.  Reads and
writes get partially independent bandwidth (duplex AXI/HBM paths), so
the schedule maximizes read+write overlap and keeps the store queue
backlogged.  Schedule: 16 tiles of [128, 3136] (half a sample each,
contiguous 1.6 MB DRAM blocks).  Loads ride TWO queues (SP HWDGE +
gpsimd SWDGE, alternating tiles) while stores ride one (ACT HWDGE):
loads keep ~2/3 of the queue presence, finish early, and the ACT store
stream runs with a deep backlog mid-kernel, then saturates the tail
with no mul-latency bubble.  Tile 14's load is split 3:1 between the
two load queues: SP otherwise finishes ~5 us after POOL (the HWDGE
store ring contends with SP more than with SWDGE), and balancing the
finish times starts the final store drain earlier.  Tiny mask inputs
load at the head of the SP ring; the embeds transpose happens on PE (a
transposed 128x4B-descriptor DMA would starve ~10 us behind bulk
traffic).  Rejected on HW: fp16 staging, smooth early-store schedules,
dual store queues, big/strided tiles, contiguous-half and
partition-half load splits (the two-read-queue interleave penalty is
SDMA-engine-internal and layout-independent).

Raw bass (no Tile): the pinned walrus codegen allows only ONE sync-wait
per compute instruction, so all cross-engine deps use standalone
wait_ge sequencer commands.

Engine plan:
  SP   (nc.sync)   - small input DMAs, then x loads (even tiles)
  POOL (nc.gpsimd) - x loads, odd tiles
  ACT  (nc.scalar) - output DMAs
  PE   (nc.tensor) - embeds transpose + projection + 2 mask transposes
  DVE  (nc.vector) - threshold search, mask build, streaming multiplies
"""

import sys

if "/opt/trn_rl_repo" not in sys.path:
    sys.path.insert(0, "/opt/trn_rl_repo")

from contextlib import ExitStack

import numpy as np

import concourse.bass as bass
from concourse import mybir
from concourse.bass_utils import run_bass_kernel_spmd

N, C, H, W = 64, 256, 56, 56
HW = H * W  # 3136
E = 16
NCORES = 8
NLOC = N // NCORES  # 8 samples per core
INDEX = 16  # ceil(C ** 0.5)
SCALE = float(C) / (C - INDEX)
F32 = mybir.dt.float32
NT = 16  # tiles: [128, HW], tile k = sample k//2, channels (k%2)*128+p
SLOTS = 14  # x-tile ring slots (12.25 KB/partition each)
SPLIT = 2352  # tile-14 free-axis split point (3:1 SP:POOL rebalance)

_NC_CACHE = {}


def _build_nc() -> bass.Bass:
    # detect_race_conditions only affects the interpreter: its raw-bass model
    # has no same-engine program-order edges, so every chained DVE op would be
    # flagged.  Cross-engine ordering is handled by the explicit sems below.
    nc = bass.Bass(detect_race_conditions=False)
    x = nc.dram_tensor("x", [NLOC * C, HW], F32, kind="ExternalInput")
    emb = nc.dram_tensor("embeds", [NLOC, E], F32, kind="ExternalInput")
    tab = nc.dram_tensor("table", [E, C], F32, kind="ExternalInput")
    out = nc.dram_tensor("out", [NLOC * C, HW], F32, kind="ExternalOutput")
    ident_d = nc.inline_tensor(np.eye(NLOC, dtype=np.float32), name="ident8")

    # row r = k*128 + p  ->  sample k//2, channel (k%2)*128 + p
    x_k = x[:, :].rearrange("(k p) f -> k p f", p=128)
    o_k = out[:, :].rearrange("(k p) f -> k p f", p=128)

    with ExitStack() as ctx:
        sb = lambda name, shape: ctx.enter_context(nc.sbuf_tensor(name, shape, F32))
        ps = lambda name, shape: ctx.enter_context(nc.psum_tensor(name, shape, F32))

        tab_s = sb("tab_s", [E, C])
        emb_s = sb("emb_s", [NLOC, E])
        embT = sb("embT", [E, NLOC])
        ident = sb("ident", [NLOC, NLOC])
        v = sb("v", [NLOC, C])
        v2 = sb("v2", [NLOC, C])
        mx = sb("mx", [NLOC, 8])
        m = sb("m", [NLOC, C])
        mA = sb("mA", [C // 2, NLOC])  # channels   0-127 x sample
        mB = sb("mB", [C // 2, NLOC])  # channels 128-255 x sample
        xbuf = [sb(f"xbuf{i}", [128, HW]) for i in range(SLOTS)]

        embT_p = ps("embT_p", [E, NLOC])
        activ_p = ps("activ_p", [NLOC, C])
        mA_p = ps("mA_p", [C // 2, NLOC])
        mB_p = ps("mB_p", [C // 2, NLOC])

        ld = ctx.enter_context(nc.semaphore("ld"))
        eb = ctx.enter_context(nc.semaphore("eb"))
        fz = ctx.enter_context(nc.semaphore("fz"))
        dv = ctx.enter_context(nc.semaphore("dv"))
        pe = ctx.enter_context(nc.semaphore("pe"))
        # per-ring-slot DMA sems: same-sem increments are serialized by the
        # slot lifecycle, so wait values are unambiguous (race-detector clean)
        xs = [ctx.enter_context(nc.semaphore(f"xs{i}")) for i in range(SLOTS)]
        ss = [ctx.enter_context(nc.semaphore(f"ss{i}")) for i in range(SLOTS)]

        block = ctx.enter_context(nc.Block())

        # Smalls first on SP: ~25 contiguous descriptors drain in a couple
        # of SDMA round-robin visits even once bulk loads queue behind them
        # (a transposed-embeds DMA would be a 128x4B descriptor spray that
        # starves for 10+ us behind bulk traffic; PE transposes instead).
        # Even tiles then load on the same SP HWDGE ring.
        @block.sync
        def _(sync):
            sync.dma_start(out=tab_s[:, :], in_=tab[:, :]).then_inc(ld, 16)
            sync.dma_start(out=emb_s[:, :], in_=emb[:, :]).then_inc(ld, 16)
            sync.dma_start(out=ident[:, :], in_=ident_d[:, :]).then_inc(ld, 16)
            for k in range(0, NT - 2, 2):
                sync.dma_start(out=xbuf[k % SLOTS][:, :], in_=x_k[k]).then_inc(
                    xs[k % SLOTS], 16
                )
            # tile 14 is split 3:1 with POOL: SP consistently finishes
            # ~5 us after POOL (the HWDGE store ring contends with SP more
            # than with SWDGE), so ~0.4 MB shifts to POOL to balance the
            # two load queues' finish times
            sync.wait_ge(ss[0], 16)  # slot free once store of tile 0 drained
            sync.dma_start(
                out=xbuf[0][:, 0:SPLIT], in_=x_k[14][:, 0:SPLIT]
            ).then_inc(xs[0], 16)

        # Odd tiles load via SWDGE so loads occupy 2 of the 3 busy DMA
        # queues (bandwidth shares follow queue counts under the SDMA
        # packet round-robin).
        @block.gpsimd
        def _(gpsimd):
            for k in range(1, NT - 1, 2):
                gpsimd.dma_start(out=xbuf[k % SLOTS][:, :], in_=x_k[k]).then_inc(
                    xs[k % SLOTS], 16
                )
            gpsimd.wait_ge(ss[0], 16)
            gpsimd.dma_start(
                out=xbuf[0][:, SPLIT:HW], in_=x_k[14][:, SPLIT:HW]
            ).then_inc(xs[0], 16)
            gpsimd.wait_ge(ss[1], 16)
            gpsimd.dma_start(out=xbuf[1][:, :], in_=x_k[15]).then_inc(xs[1], 16)

        @block.tensor
        def _(tensor):
            tensor.wait_ge(ld, 48)  # tab_s + emb_s + ident resident
            tensor.matmul(
                embT_p[:, :], emb_s[:, :], ident[:, :], start=True, stop=True
            ).then_inc(pe, 1)
            tensor.wait_ge(eb, 1)  # embT copied to SBUF
            tensor.matmul(
                activ_p[:, :], embT[:, :], tab_s[:, :], start=True, stop=True
            ).then_inc(pe, 1)
            tensor.wait_ge(dv, 1)  # mask row built
            tensor.matmul(
                mA_p[:, :], m[:, 0 : C // 2], ident[:, :], start=True, stop=True
            ).then_inc(pe, 1)
            tensor.matmul(
                mB_p[:, :], m[:, C // 2 : C], ident[:, :], start=True, stop=True
            ).then_inc(pe, 1)

        # The 16 smallest of activ == the 16 largest of v = -activ.  DVE's
        # max (top-8 per partition) + match_replace (zap those 8) drop them
        # in two rounds; surviving lanes keep their value, zapped lanes hold
        # MINV, so the mask is one compare against an immediate.  No
        # data-dependent scalar operands anywhere: TensorScalarPtr fetches
        # its scalar at sequencer dispatch (ahead of the DVE pipe), so only
        # mA/mB -- real pointer operands of the streaming muls -- need a
        # sem fence.
        MINV = -1.0e30

        @block.vector
        def _(vector):
            vector.wait_ge(pe, 1)
            vector.tensor_copy(embT[:, :], embT_p[:, :]).then_inc(eb, 1)
            vector.wait_ge(pe, 2)
            vector.tensor_scalar_mul(v[:, :], activ_p[:, :], -1.0)
            # match_replace prefetches its 8-value table at dispatch, ahead
            # of the DVE pipe -- fence each max before consuming it
            vector.max(mx[:, :], v[:, :]).then_inc(fz, 1)
            vector.wait_ge(fz, 1)
            vector.match_replace(
                out=v2[:, :], in_to_replace=mx[:, :], in_values=v[:, :],
                imm_value=MINV,
            )
            vector.max(mx[:, :], v2[:, :]).then_inc(fz, 1)
            vector.wait_ge(fz, 2)
            vector.match_replace(
                out=v2[:, :], in_to_replace=mx[:, :], in_values=v2[:, :],
                imm_value=MINV,
            )
            # keep[c] <=> v2[c] != MINV ; mask = keep * SCALE
            # (immediate compare: real values are > MINV/2)
            vector.tensor_scalar(
                out=m[:, :],
                in0=v2[:, :],
                scalar1=MINV / 2,
                scalar2=SCALE,
                op0=mybir.AluOpType.is_ge,
                op1=mybir.AluOpType.mult,
            ).then_inc(dv, 1)
            vector.wait_ge(pe, 4)
            vector.tensor_copy(mA[:, :], mA_p[:, :])
            vector.tensor_copy(mB[:, :], mB_p[:, :]).then_inc(dv, 1)
            vector.wait_ge(dv, 2)  # mA/mB committed before mul ptr-fetches
            for k in range(NT):
                # slot 0 second pass needs 3 incs: load 0 + both tile-14 halves
                vector.wait_ge(xs[k % SLOTS], 48 if k == 14 else 16 * (k // SLOTS + 1))
                mcol = (mA if k % 2 == 0 else mB)[:, k // 2 : k // 2 + 1]
                vector.tensor_scalar_mul(
                    xbuf[k % SLOTS][:, :], xbuf[k % SLOTS][:, :], mcol
                ).then_inc(dv, 1)

        DV_BASE = 2  # dv value once masks + mA/mB copies are done

        @block.scalar
        def _(scalar):
            for k in range(NT):
                scalar.wait_ge(dv, DV_BASE + (k + 1))  # mul of tile k done
                scalar.dma_start(out=o_k[k], in_=xbuf[k % SLOTS][:, :]).then_inc(
                    ss[k % SLOTS], 16
                )

    return nc


def _get_nc() -> bass.Bass:
    if "nc" not in _NC_CACHE:
        _NC_CACHE["nc"] = _build_nc()
    return _NC_CACHE["nc"]


def _in_maps(x, embeds, table):
    x = np.ascontiguousarray(np.asarray(x, dtype=np.float32))
    embeds = np.ascontiguousarray(np.asarray(embeds, dtype=np.float32))
    table = np.ascontiguousarray(np.asarray(table, dtype=np.float32))
    maps = []
    for i in range(NCORES):
        maps.append(
            {
                "x": x[i * NLOC : (i + 1) * NLOC].reshape(NLOC * C, HW),
                "embeds": embeds[i * NLOC : (i + 1) * NLOC],
                "table": table,
            }
        )
    return maps


def kernel(x, embeds, table):
    nc = _get_nc()
    res = run_bass_kernel_spmd(nc, _in_maps(x, embeds, table), list(range(NCORES)))
    shards = [
        np.asarray(res.results[i]["out"]).reshape(NLOC, C, H, W)
        for i in range(NCORES)
    ]
    return np.concatenate(shards, axis=0)


def kernel_profiled(x, embeds, table, **trace_kwargs):
    """Same as kernel() but with NTFF tracing; returns (output, BassKernelResults)."""
    nc = _get_nc()
    res = run_bass_kernel_spmd(
        nc, _in_maps(x, embeds, table), list(range(NCORES)), trace=True, **trace_kwargs
    )
    shards = [
        np.asarray(res.results[i]["out"]).reshape(NLOC, C, H, W)
        for i in range(NCORES)
    ]
    return np.concatenate(shards, axis=0), res
